# revision 22
# baseline (speedup 1.0000x reference)
"""Causal multi-head attention (B=8, S=1024, E=1024, H=16, D=64) on 8 TRN2 NeuronCores.

Strategy: pure data parallelism over the batch — one batch element per core,
full weights replicated, zero collectives. Per-core attention with TRANSPOSED
scores (k on partitions), which eliminates the DMA-crossbar transpose of the
attention matrix entirely:

  - host passes x[b] pre-transposed (xT = [E, S]) and weights cast to bf16;
    all matmuls run in bf16 with fp32 PSUM accumulation.
  - V = [s, pair*(64+64+64)] is projected first; per pair the layout is
    [v_h0 | ones | v_h1] so head h0's attn@V stationary [v_h0|ones] puts the
    softmax row-sums (replicated 64x) in PSUM rows 64:128, and h1's
    [ones|v_h1] puts them in rows 0:64 — the sums come free with the matmul.
  - QT/KT = [head*64+d, s] chunks are projected just-in-time per head-pair;
    their PSUM->SBUF copies run on ACT so DVE stays off that chain.
  - scoresT[k, q] per (head, k-chunk): lhsT = KT chunk (stationary), rhs
    streams all causal q columns (up to 512/instr). The causal mask for the
    diagonal block is INJECTED INTO PSUM by an identity matmul (start=True)
    that the scores matmul then accumulates onto (start=False) — masking
    costs one extra 128-col PE matmul, no DVE/ACT work.
  - exp on ACT reads the PSUM scores and writes unnormalized attnT tiles
    (bf16, exact causal widths, no transpose needed).
  - attn@V accumulates outT = [d, q] over k-chunks with wide streams;
    normalization is deferred: one reciprocal_approx_fast + one [64, S]
    multiply per head (vs. normalizing the full [128, 4608] attn matrix).
  - out = ct.T @ Wo + bias-add on DVE from a host-broadcast bo tile.
"""

import numpy as np
import ml_dtypes

B, S, E = 8, 1024, 1024
H, D = 16, 64
HD = H * D
NCORES = 8
P = 128
NCH = E // P  # 8 contraction chunks
NT = S // P  # 8 q tiles
NPAIR = H // 2
SCALE = 1.0 / np.sqrt(D)
BF16 = ml_dtypes.bfloat16

_graph_cache = {}


def _patch_tile_drain():
    """The walrus build in this container only allows a single sync wait on the
    TPB_CTRL Drain that TileContext emits at kernel tail. Spread the end-of-
    kernel waits across SP nops (one wait each) before the drain instead."""
    import concourse.tile as tile
    import concourse.mybir as mybir
    from concourse.vector_clock import ScopedClock

    if getattr(tile.TileContext, "_drain_patched", False):
        return

    def _drain_and_barrier(self, tick_clock, wait_clock):
        nop0 = self.nc.sync.nop(nofuse=True)
        wait_clock.add_sem_waits(
            nop0.ins, ScopedClock({None: tick_clock.global_clock})
        )
        waits = list(nop0.ins.sync_info.on_wait) if nop0.ins.sync_info else []
        if len(waits) > 1:
            nop0.ins.sync_info = mybir.SyncInfo(
                on_wait=waits[:1], on_update=list(nop0.ins.sync_info.on_update)
            )
            for w in waits[1:]:
                n = self.nc.sync.nop(nofuse=True)
                n.ins.sync_info = mybir.SyncInfo(on_wait=[w], on_update=[])
        self.nc.sync.drain()
        self.nc.all_engine_barrier()
        assert self.sems is not None
        popped = self.nc._tile_sem_poison_stack.pop()
        assert popped is self._sem_poison
        self.nc.clear_and_free_semaphores(list(self.sems.allocated().values()))
        self.nc.all_engine_barrier()

    tile.TileContext._drain_and_barrier = _drain_and_barrier
    tile.TileContext._drain_patched = True


def _split_waits(nc, maxw=1):
    """This container's walrus build rejects instructions carrying more than
    one sync wait. Move excess waits onto same-engine NoOps inserted directly
    before the instruction (sequencer order makes this semantically identical.
    Safe for a straight-line Tile program: waits only reference predecessors
    in the dependency DAG, so stalling the sequencer earlier cannot deadlock."""
    import concourse.mybir as mybir

    ctr = [0]
    for fn in nc.m.functions:
        for bb in fn.blocks:
            insts = bb.instructions
            out = []
            changed = False
            for inst in insts:
                si = getattr(inst, "sync_info", None)
                if si is not None and len(si.on_wait) > maxw:
                    waits = list(si.on_wait)
                    keep = waits[:maxw]
                    rest = waits[maxw:]
                    for i in range(0, len(rest), maxw):
                        n = mybir.InstNoOp(
                            name=f"waitsplit-{ctr[0]}", ins=[], outs=[]
                        )
                        ctr[0] += 1
                        n.engine = inst.engine
                        n.sync_info = mybir.SyncInfo(
                            on_wait=rest[i : i + maxw], on_update=[]
                        )
                        out.append(n)
                    inst.sync_info = mybir.SyncInfo(
                        on_wait=keep, on_update=list(si.on_update)
                    )
                    changed = True
                out.append(inst)
            if changed:
                bb.instructions = out
    return nc


def _build():
    import concourse.bass as bass
    import concourse.tile as tile
    import concourse.mybir as mybir

    f32 = mybir.dt.float32
    bf16 = mybir.dt.bfloat16

    nc = bass.Bass("TRN2", target_bir_lowering=False, debug=False, num_devices=NCORES)
    xT_ext = nc.declare_dram_parameter("xT", [E, S], bf16, isOutput=False)
    wq_ext = nc.declare_dram_parameter("wq", [E, HD], bf16, isOutput=False)
    wk_ext = nc.declare_dram_parameter("wk", [E, HD], bf16, isOutput=False)
    wv_ext = nc.declare_dram_parameter("wv", [E, HD], bf16, isOutput=False)
    wo_ext = nc.declare_dram_parameter("wo", [HD, E], bf16, isOutput=False)
    bo_ext = nc.declare_dram_parameter("bo", [P, E], f32, isOutput=False)
    out_ext = nc.declare_dram_parameter("out", [S, E], bf16, isOutput=True)

    EXP = mybir.ActivationFunctionType.Exp
    LOG = mybir.ActivationFunctionType.Ln

    with tile.TileContext(nc) as tc:
        with (
            tc.tile_pool(name="consts", bufs=1) as consts,
            tc.tile_pool(name="xt", bufs=NCH) as xt_pool,
            tc.tile_pool(name="w3", bufs=3 * NCH) as w3_pool,
            tc.tile_pool(name="wo", bufs=NCH) as wo_pool,
            tc.tile_pool(name="vp", bufs=NT) as v_pool,
            tc.tile_pool(name="qt", bufs=2) as qt_pool,
            tc.tile_pool(name="kt", bufs=2) as kt_pool,
            tc.tile_pool(name="ct", bufs=NCH) as ct_pool,
            tc.tile_pool(name="at", bufs=10) as at_pool,
            tc.tile_pool(name="rbc", bufs=4) as rbc_pool,
            tc.tile_pool(name="outp", bufs=2) as out_pool,
            tc.tile_pool(name="psA", bufs=4, space="PSUM") as psA,
            tc.tile_pool(name="psB", bufs=2, space="PSUM") as psB,
        ):
            # constants: transposed binary causal mask (1 where q >= k,
            # 0 strictly below the diagonal), broadcast bias
            binmaskT = consts.tile([P, P], bf16, tag="binmaskT")
            nc.gpsimd.memset(binmaskT[:], 1.0)
            nc.gpsimd.affine_select(
                out=binmaskT[:],
                in_=binmaskT[:],
                compare_op=mybir.AluOpType.is_ge,
                fill=0.0,
                base=0,
                # keep 1 where (q - k) >= 0, fill 0 where q < k
                pattern=[[1, P]],
                channel_multiplier=-1,
            )
            bo_sb = consts.tile([P, E], f32, tag="bo")

            xt_sb = [xt_pool.tile([P, S], bf16, tag="xt", name=f"xt{_}") for _ in range(NCH)]
            wq_sb = [w3_pool.tile([P, HD], bf16, tag="w3", name=f"wq{_}") for _ in range(NCH)]
            wk_sb = [w3_pool.tile([P, HD], bf16, tag="w3", name=f"wk{_}") for _ in range(NCH)]
            wv_sb = [w3_pool.tile([P, HD], bf16, tag="w3", name=f"wv{_}") for _ in range(NCH)]
            wo_sb = [wo_pool.tile([P, E], bf16, tag="wo", name=f"wo{_}") for _ in range(NCH)]
            # x + Wv first (V projection is the critical path at start),
            # then Wq/Wk (needed from pair 0 on), Wo last.
            for c in range(NCH):
                rows = slice(c * P, (c + 1) * P)
                nc.sync.dma_start(out=xt_sb[c][:], in_=xT_ext[rows, :])
                nc.scalar.dma_start(out=wv_sb[c][:], in_=wv_ext[rows, :])
            for c in range(NCH):
                rows = slice(c * P, (c + 1) * P)
                nc.sync.dma_start(out=wq_sb[c][:], in_=wq_ext[rows, :])
                nc.scalar.dma_start(out=wk_sb[c][:], in_=wk_ext[rows, :])
            for c in range(NCH):
                rows = slice(c * P, (c + 1) * P)
                eng = nc.sync if c % 2 == 0 else nc.scalar
                eng.dma_start(out=wo_sb[c][:], in_=wo_ext[rows, :])
            nc.scalar.dma_start(out=bo_sb[:], in_=bo_ext[:])

            # ---------- V projection; per-pair layout [v_h0 | ones | v_h1] ----
            v_sb = [
                v_pool.tile([P, NPAIR, 3 * D], bf16, tag="v", name=f"v{_}")
                for _ in range(NT)
            ]
            for t in range(NT):
                for hf in range(2):
                    ps = psA.tile([P, 512], f32, tag="ps", name=f"vps{t}_{hf}")
                    for k in range(NCH):
                        nc.tensor.matmul(
                            ps[:],
                            lhsT=xt_sb[k][:, t * P : (t + 1) * P],
                            rhs=wv_sb[k][:, hf * 512 : (hf + 1) * 512],
                            start=(k == 0),
                            stop=(k == NCH - 1),
                        )
                    ps3 = ps[:].rearrange("p (g c) -> p g c", c=2 * D)
                    gsl = slice(hf * 4, (hf + 1) * 4)
                    nc.vector.tensor_copy(v_sb[t][:, gsl, 0:D], ps3[:, :, 0:D])
                    nc.vector.tensor_copy(
                        v_sb[t][:, gsl, 2 * D : 3 * D], ps3[:, :, D : 2 * D]
                    )
                nc.gpsimd.memset(v_sb[t][:, :, D : 2 * D], 1.0)

            ct_sb = [ct_pool.tile([P, S], bf16, tag="ct", name=f"ct{_}") for _ in range(NCH)]

            # ---------- per head-pair attention ----------
            def qk_alloc(c):
                qt = qt_pool.tile([P, S], bf16, tag="qt", name=f"qt{c}")
                kt = kt_pool.tile([P, S], bf16, tag="kt", name=f"kt{c}")
                return qt, kt

            def qk_chunk(c, qt, kt, j):
                # one of the four (projection, s-half) psum tiles: 8 matmuls
                # + its copy — sized to interleave between kc iterations
                w_sb, dst = ((wq_sb, qt), (wk_sb, kt))[j // 2]
                hf = j % 2
                ps = psA.tile([P, 512], f32, tag="ps", name=f"qk{c}_{j}")
                for k in range(NCH):
                    nc.tensor.matmul(
                        ps[:],
                        lhsT=w_sb[k][:, c * P : (c + 1) * P],
                        rhs=xt_sb[k][:, hf * 512 : (hf + 1) * 512],
                        start=(k == 0),
                        stop=(k == NCH - 1),
                    )
                nc.vector.tensor_copy(dst[:, hf * 512 : (hf + 1) * 512], ps[:])

            # Cross-pair software pipeline: one global stream of (pair, kc)
            # iterations; attn@V lags scores by LAG iterations and flows
            # across pair boundaries so the PE never drains (a drained PE
            # drops to half clock and takes microseconds to ramp back).
            LAG = 2
            state = {}

            def scores_iter(c, kc):
                st = state[c]
                qt, kt = st["qt"], st["kt"]
                W = S - kc * P
                scs = []
                for hl in range(2):
                    d0, d1 = hl * D, (hl + 1) * D
                    kt_sl = kt[d0:d1, kc * P : (kc + 1) * P]
                    if W > 512:
                        scA = psA.tile([P, 512], f32, tag="ps", name=f"scA{c}_{kc}_{hl}")
                        scB = psA.tile([P, 512], f32, tag="ps", name=f"scB{c}_{kc}_{hl}")
                        nc.tensor.matmul(
                            scA[:],
                            lhsT=kt_sl,
                            rhs=qt[d0:d1, kc * P : kc * P + 512],
                            start=True, stop=True,
                        )
                        nc.tensor.matmul(
                            scB[:, 0 : W - 512],
                            lhsT=kt_sl,
                            rhs=qt[d0:d1, kc * P + 512 : S],
                            start=True, stop=True,
                        )
                        scs.append((scA, scB))
                    else:
                        scA = psA.tile([P, 512], f32, tag="ps", name=f"scA{c}_{kc}_{hl}")
                        nc.tensor.matmul(
                            scA[:, 0:W],
                            lhsT=kt_sl,
                            rhs=qt[d0:d1, kc * P : S],
                            start=True, stop=True,
                        )
                        scs.append((scA, None))
                for hl in range(2):
                    scA, scB = scs[hl]
                    at = at_pool.tile([P, W], bf16, tag="at", name=f"at{c}_{kc}_{hl}")
                    if scB is not None:
                        nc.scalar.activation(
                            at[:, 0:512], scA[:], EXP, bias=0.0, scale=float(SCALE)
                        )
                        nc.scalar.activation(
                            at[:, 512:W], scB[:, 0 : W - 512], EXP,
                            bias=0.0, scale=float(SCALE),
                        )
                    else:
                        nc.scalar.activation(
                            at[:, 0:W], scA[:, 0:W], EXP, bias=0.0, scale=float(SCALE)
                        )
                    st["at"][(hl, kc)] = at
                for hl in range(2):
                    at = st["at"][(hl, kc)]
                    # zero the strictly-lower triangle of the diagonal block
                    nc.vector.tensor_mul(at[:, 0:P], at[:, 0:P], binmaskT[:])

            def attn_v(c, hl, kc):
                st = state[c]
                if "po" not in st:
                    st["po"] = [
                        psB.tile([P, S], f32, tag="po", name=f"po{c}_{h}")
                        for h in range(2)
                    ]
                W = S - kc * P
                at = st["at"].pop((hl, kc))
                # h0 stationary = [v_h0 | ones]; h1 = [ones | v_h1]
                lhsT = (
                    v_sb[kc][:, c, 0 : 2 * D]
                    if hl == 0
                    else v_sb[kc][:, c, D : 3 * D]
                )
                p = st["po"][hl]
                if kc == 0:
                    nc.tensor.matmul(
                        p[:, 0:512], lhsT=lhsT, rhs=at[:, 0:512],
                        start=True, stop=False,
                    )
                    nc.tensor.matmul(
                        p[:, 512:1024], lhsT=lhsT, rhs=at[:, 512:1024],
                        start=True, stop=False,
                    )
                elif kc < 4:
                    # split on the aT exp_a/exp_b boundary (col 512) so
                    # each matmul depends on exactly one exp instruction
                    nc.tensor.matmul(
                        p[:, kc * P : 512], lhsT=lhsT, rhs=at[:, 0 : 512 - kc * P],
                        start=False, stop=(kc == 3),
                    )
                    nc.tensor.matmul(
                        p[:, 512 : 512 + kc * P],
                        lhsT=lhsT,
                        rhs=at[:, 512 - kc * P : 512],
                        start=False, stop=False,
                    )
                    nc.tensor.matmul(
                        p[:, 512 + kc * P : 1024],
                        lhsT=lhsT,
                        rhs=at[:, 512:W],
                        start=False, stop=False,
                    )
                else:
                    nc.tensor.matmul(
                        p[:, kc * P : 1024], lhsT=lhsT, rhs=at[:, 0:W],
                        start=False, stop=(kc == 7),
                    )

            def normalize(c):
                # r = exp(-ln(sums)) on ACT with ln reading the replicated
                # row-sums straight from PSUM, then outT rows * r on DVE
                po = state[c]["po"]
                rr = rbc_pool.tile([P, S], f32, tag="rr", name=f"rr{c}")
                nc.scalar.activation(rr[D : 2 * D, :], po[0][D : 2 * D, :], LOG)
                nc.scalar.activation(rr[0:D, :], po[1][0:D, :], LOG)
                nc.scalar.activation(rr[:], rr[:], EXP, bias=0.0, scale=-1.0)
                nc.vector.tensor_mul(ct_sb[c][0:D, :], po[0][0:D, :], rr[D : 2 * D, :])
                nc.vector.tensor_mul(
                    ct_sb[c][D : 2 * D, :], po[1][D : 2 * D, :], rr[0:D, :]
                )
                del state[c]

            qt_next = qk_alloc(0)
            for j in range(4):
                qk_chunk(0, qt_next[0], qt_next[1], j)

            G = NPAIR * NT
            for g in range(G + LAG):
                if g < G:
                    c, kc = divmod(g, NT)
                    if kc == 0:
                        state[c] = dict(qt=qt_next[0], kt=qt_next[1], at={})
                    scores_iter(c, kc)
                    if c + 1 < NPAIR:
                        if kc == 2:
                            qt_next = qk_alloc(c + 1)
                        if 2 <= kc <= 5:
                            qk_chunk(c + 1, qt_next[0], qt_next[1], kc - 2)
                if g >= LAG:
                    c2, kc2 = divmod(g - LAG, NT)
                    attn_v(c2, 0, kc2)
                    attn_v(c2, 1, kc2)
                    if kc2 == NT - 1:
                        normalize(c2)

            # ---------- output projection + bias ----------
            for t in range(NT):
                osb = out_pool.tile([P, E], bf16, tag="out", name=f"ou{t}")
                for hf in range(2):
                    op = psA.tile([P, 512], f32, tag="ps", name=f"op{t}_{hf}")
                    for cc in range(NCH):
                        nc.tensor.matmul(
                            op[:],
                            lhsT=ct_sb[cc][:, t * P : (t + 1) * P],
                            rhs=wo_sb[cc][:, hf * 512 : (hf + 1) * 512],
                            start=(cc == 0),
                            stop=(cc == NCH - 1),
                        )
                    nc.vector.tensor_add(
                        osb[:, hf * 512 : (hf + 1) * 512],
                        op[:],
                        bo_sb[:, hf * 512 : (hf + 1) * 512],
                    )
                eng = nc.sync if t % 2 == 0 else nc.scalar
                eng.dma_start(out=out_ext[t * P : (t + 1) * P, :], in_=osb[:])

    return _split_waits(nc)


def _get_graph():
    if "nc" not in _graph_cache:
        _patch_tile_drain()
        _graph_cache["nc"] = _build()
    return _graph_cache["nc"]


def _prep_inputs(x, Wq, Wk, Wv, Wo, bo):
    xT = np.ascontiguousarray(np.transpose(np.asarray(x, np.float32), (0, 2, 1)))
    xT = xT.astype(BF16)
    wq = np.ascontiguousarray(
        np.asarray(Wq, np.float32).transpose(1, 0, 2).reshape(E, HD)
    ).astype(BF16)
    wk = np.ascontiguousarray(
        np.asarray(Wk, np.float32).transpose(1, 0, 2).reshape(E, HD)
    ).astype(BF16)
    wv = np.ascontiguousarray(
        np.asarray(Wv, np.float32).transpose(1, 0, 2).reshape(E, HD)
    ).astype(BF16)
    wo = np.ascontiguousarray(np.asarray(Wo, np.float32)).astype(BF16)
    bo_t = np.ascontiguousarray(
        np.tile(np.asarray(bo, np.float32)[None, :], (P, 1))
    )
    return [
        dict(
            xT=np.ascontiguousarray(xT[b]),
            wq=wq,
            wk=wk,
            wv=wv,
            wo=wo,
            bo=bo_t,
        )
        for b in range(B)
    ]


def _run(in_maps, **kw):
    from concourse.bass_utils import run_bass_kernel_spmd

    nc = _get_graph()
    return run_bass_kernel_spmd(nc, in_maps, core_ids=list(range(NCORES)), **kw)


def kernel(x, Wq, Wk, Wv, Wo, bo):
    res = _run(_prep_inputs(x, Wq, Wk, Wv, Wo, bo))
    return np.stack(
        [np.asarray(res.results[b]["out"], np.float32) for b in range(B)], axis=0
    )


# revision 24
# speedup vs baseline: 1.2127x; 1.2127x over previous
"""Causal multi-head attention (B=8, S=1024, E=1024, H=16, D=64) on 8 TRN2 NeuronCores.

Strategy: pure data parallelism over the batch — one batch element per core,
full weights replicated, zero collectives. Per-core attention with TRANSPOSED
scores (k on partitions), which eliminates the DMA-crossbar transpose of the
attention matrix entirely:

  - host passes x[b] pre-transposed (xT = [E, S]) and weights cast to bf16;
    all matmuls run in bf16 with fp32 PSUM accumulation.
  - V = [s, pair*(64+64+64)] is projected first; per pair the layout is
    [v_h0 | ones | v_h1] so head h0's attn@V stationary [v_h0|ones] puts the
    softmax row-sums (replicated 64x) in PSUM rows 64:128, and h1's
    [ones|v_h1] puts them in rows 0:64 — the sums come free with the matmul.
  - QT/KT = [head*64+d, s] chunks are projected just-in-time per head-pair;
    their PSUM->SBUF copies run on ACT so DVE stays off that chain.
  - scoresT[k, q] per (head, k-chunk): lhsT = KT chunk (stationary), rhs
    streams all causal q columns (up to 512/instr). The causal mask for the
    diagonal block is INJECTED INTO PSUM by an identity matmul (start=True)
    that the scores matmul then accumulates onto (start=False) — masking
    costs one extra 128-col PE matmul, no DVE/ACT work.
  - exp on ACT reads the PSUM scores and writes unnormalized attnT tiles
    (bf16, exact causal widths, no transpose needed).
  - attn@V accumulates outT = [d, q] over k-chunks with wide streams;
    normalization is deferred: one reciprocal_approx_fast + one [64, S]
    multiply per head (vs. normalizing the full [128, 4608] attn matrix).
  - out = ct.T @ Wo + bias-add on DVE from a host-broadcast bo tile.
"""

import numpy as np
import ml_dtypes

B, S, E = 8, 1024, 1024
H, D = 16, 64
HD = H * D
NCORES = 8
P = 128
NCH = E // P  # 8 contraction chunks
NT = S // P  # 8 q tiles
NPAIR = H // 2
SCALE = 1.0 / np.sqrt(D)
BF16 = ml_dtypes.bfloat16

_graph_cache = {}


def _patch_tile_drain():
    """The walrus build in this container only allows a single sync wait on the
    TPB_CTRL Drain that TileContext emits at kernel tail. Spread the end-of-
    kernel waits across SP nops (one wait each) before the drain instead."""
    import concourse.tile as tile
    import concourse.mybir as mybir
    from concourse.vector_clock import ScopedClock

    if getattr(tile.TileContext, "_drain_patched", False):
        return

    def _drain_and_barrier(self, tick_clock, wait_clock):
        nop0 = self.nc.sync.nop(nofuse=True)
        wait_clock.add_sem_waits(
            nop0.ins, ScopedClock({None: tick_clock.global_clock})
        )
        waits = list(nop0.ins.sync_info.on_wait) if nop0.ins.sync_info else []
        if len(waits) > 1:
            nop0.ins.sync_info = mybir.SyncInfo(
                on_wait=waits[:1], on_update=list(nop0.ins.sync_info.on_update)
            )
            for w in waits[1:]:
                n = self.nc.sync.nop(nofuse=True)
                n.ins.sync_info = mybir.SyncInfo(on_wait=[w], on_update=[])
        self.nc.sync.drain()
        self.nc.all_engine_barrier()
        assert self.sems is not None
        popped = self.nc._tile_sem_poison_stack.pop()
        assert popped is self._sem_poison
        self.nc.clear_and_free_semaphores(list(self.sems.allocated().values()))
        self.nc.all_engine_barrier()

    tile.TileContext._drain_and_barrier = _drain_and_barrier
    tile.TileContext._drain_patched = True


def _split_waits(nc, maxw=1):
    """This container's walrus build rejects instructions carrying more than
    one sync wait. Move excess waits onto same-engine NoOps inserted directly
    before the instruction (sequencer order makes this semantically identical.
    Safe for a straight-line Tile program: waits only reference predecessors
    in the dependency DAG, so stalling the sequencer earlier cannot deadlock."""
    import concourse.mybir as mybir

    ctr = [0]
    for fn in nc.m.functions:
        for bb in fn.blocks:
            insts = bb.instructions
            out = []
            changed = False
            for inst in insts:
                si = getattr(inst, "sync_info", None)
                if si is not None and len(si.on_wait) > maxw:
                    waits = list(si.on_wait)
                    keep = waits[:maxw]
                    rest = waits[maxw:]
                    for i in range(0, len(rest), maxw):
                        n = mybir.InstNoOp(
                            name=f"waitsplit-{ctr[0]}", ins=[], outs=[]
                        )
                        ctr[0] += 1
                        n.engine = inst.engine
                        n.sync_info = mybir.SyncInfo(
                            on_wait=rest[i : i + maxw], on_update=[]
                        )
                        out.append(n)
                    inst.sync_info = mybir.SyncInfo(
                        on_wait=keep, on_update=list(si.on_update)
                    )
                    changed = True
                out.append(inst)
            if changed:
                bb.instructions = out
    return nc


def _build():
    import concourse.bass as bass
    import concourse.tile as tile
    import concourse.mybir as mybir

    f32 = mybir.dt.float32
    bf16 = mybir.dt.bfloat16

    nc = bass.Bass("TRN2", target_bir_lowering=False, debug=False, num_devices=NCORES)
    xT_ext = nc.declare_dram_parameter("xT", [E, S], bf16, isOutput=False)
    wq_ext = nc.declare_dram_parameter("wq", [E, HD], bf16, isOutput=False)
    wk_ext = nc.declare_dram_parameter("wk", [E, HD], bf16, isOutput=False)
    wv_ext = nc.declare_dram_parameter("wv", [E, HD], bf16, isOutput=False)
    wo_ext = nc.declare_dram_parameter("wo", [HD, E], bf16, isOutput=False)
    bo_ext = nc.declare_dram_parameter("bo", [P, E], f32, isOutput=False)
    out_ext = nc.declare_dram_parameter("out", [S, E], bf16, isOutput=True)

    EXP = mybir.ActivationFunctionType.Exp
    LOG = mybir.ActivationFunctionType.Ln

    with tile.TileContext(nc) as tc:
        with (
            tc.tile_pool(name="consts", bufs=1) as consts,
            tc.tile_pool(name="xt", bufs=NCH) as xt_pool,
            tc.tile_pool(name="w3", bufs=3 * NCH) as w3_pool,
            tc.tile_pool(name="wo", bufs=NCH) as wo_pool,
            tc.tile_pool(name="vp", bufs=NT) as v_pool,
            tc.tile_pool(name="qt", bufs=2) as qt_pool,
            tc.tile_pool(name="kt", bufs=2) as kt_pool,
            tc.tile_pool(name="ct", bufs=NCH) as ct_pool,
            tc.tile_pool(name="at", bufs=8) as at_pool,
            tc.tile_pool(name="rbc", bufs=4) as rbc_pool,
            tc.tile_pool(name="outp", bufs=2) as out_pool,
            tc.tile_pool(name="psA", bufs=4, space="PSUM") as psA,
            tc.tile_pool(name="psB", bufs=2, space="PSUM") as psB,
        ):
            # constants: transposed binary causal mask (1 where q >= k,
            # 0 strictly below the diagonal), broadcast bias
            binmaskT = consts.tile([P, P], bf16, tag="binmaskT")
            nc.gpsimd.memset(binmaskT[:], 1.0)
            nc.gpsimd.affine_select(
                out=binmaskT[:],
                in_=binmaskT[:],
                compare_op=mybir.AluOpType.is_ge,
                fill=0.0,
                base=0,
                # keep 1 where (q - k) >= 0, fill 0 where q < k
                pattern=[[1, P]],
                channel_multiplier=-1,
            )
            bo_sb = consts.tile([P, E], f32, tag="bo")

            xt_sb = [xt_pool.tile([P, S], bf16, tag="xt", name=f"xt{_}") for _ in range(NCH)]
            wq_sb = [w3_pool.tile([P, HD], bf16, tag="w3", name=f"wq{_}") for _ in range(NCH)]
            wk_sb = [w3_pool.tile([P, HD], bf16, tag="w3", name=f"wk{_}") for _ in range(NCH)]
            wv_sb = [w3_pool.tile([P, HD], bf16, tag="w3", name=f"wv{_}") for _ in range(NCH)]
            wo_sb = [wo_pool.tile([P, E], bf16, tag="wo", name=f"wo{_}") for _ in range(NCH)]
            # x + Wv first (V projection is the critical path at start),
            # then Wq/Wk (needed from pair 0 on), Wo last.
            for c in range(NCH):
                rows = slice(c * P, (c + 1) * P)
                nc.sync.dma_start(out=xt_sb[c][:], in_=xT_ext[rows, :])
                nc.scalar.dma_start(out=wv_sb[c][:], in_=wv_ext[rows, :])
            for c in range(NCH):
                rows = slice(c * P, (c + 1) * P)
                nc.sync.dma_start(out=wq_sb[c][:], in_=wq_ext[rows, :])
                nc.scalar.dma_start(out=wk_sb[c][:], in_=wk_ext[rows, :])
            for c in range(NCH):
                rows = slice(c * P, (c + 1) * P)
                eng = nc.sync if c % 2 == 0 else nc.scalar
                eng.dma_start(out=wo_sb[c][:], in_=wo_ext[rows, :])
            nc.scalar.dma_start(out=bo_sb[:], in_=bo_ext[:])

            # ---------- V projection; per-pair layout [v_h0 | ones | v_h1] ----
            v_sb = [
                v_pool.tile([P, NPAIR, 3 * D], bf16, tag="v", name=f"v{_}")
                for _ in range(NT)
            ]
            for t in range(NT):
                for hf in range(2):
                    ps = psA.tile([P, 512], f32, tag="ps", name=f"vps{t}_{hf}")
                    for k in range(NCH):
                        nc.tensor.matmul(
                            ps[:],
                            lhsT=xt_sb[k][:, t * P : (t + 1) * P],
                            rhs=wv_sb[k][:, hf * 512 : (hf + 1) * 512],
                            start=(k == 0),
                            stop=(k == NCH - 1),
                        )
                    ps3 = ps[:].rearrange("p (g c) -> p g c", c=2 * D)
                    gsl = slice(hf * 4, (hf + 1) * 4)
                    nc.vector.tensor_copy(v_sb[t][:, gsl, 0:D], ps3[:, :, 0:D])
                    nc.vector.tensor_copy(
                        v_sb[t][:, gsl, 2 * D : 3 * D], ps3[:, :, D : 2 * D]
                    )
                nc.gpsimd.memset(v_sb[t][:, :, D : 2 * D], 1.0)

            ct_sb = [ct_pool.tile([P, S], bf16, tag="ct", name=f"ct{_}") for _ in range(NCH)]

            # ---------- per head-pair attention ----------
            def qk_alloc(c):
                qt = qt_pool.tile([P, S], bf16, tag="qt", name=f"qt{c}")
                kt = kt_pool.tile([P, S], bf16, tag="kt", name=f"kt{c}")
                return qt, kt

            def qk_chunk(c, qt, kt, j):
                # one of the four (projection, s-half) psum tiles: 8 matmuls
                # + its copy — sized to interleave between kc iterations
                w_sb, dst = ((wq_sb, qt), (wk_sb, kt))[j // 2]
                hf = j % 2
                ps = psA.tile([P, 512], f32, tag="ps", name=f"qk{c}_{j}")
                for k in range(NCH):
                    nc.tensor.matmul(
                        ps[:],
                        lhsT=w_sb[k][:, c * P : (c + 1) * P],
                        rhs=xt_sb[k][:, hf * 512 : (hf + 1) * 512],
                        start=(k == 0),
                        stop=(k == NCH - 1),
                    )
                nc.vector.tensor_copy(dst[:, hf * 512 : (hf + 1) * 512], ps[:])

            # Cross-pair software pipeline: one global stream of (pair, kc)
            # iterations; attn@V lags scores by LAG iterations and flows
            # across pair boundaries so the PE never drains (a drained PE
            # drops to half clock and takes microseconds to ramp back).
            LAG = 2
            state = {}

            def scores_iter(c, kc):
                st = state[c]
                qt, kt = st["qt"], st["kt"]
                W = S - kc * P
                scs = []
                for hl in range(2):
                    d0, d1 = hl * D, (hl + 1) * D
                    kt_sl = kt[d0:d1, kc * P : (kc + 1) * P]
                    if W > 512:
                        scA = psA.tile([P, 512], f32, tag="ps", name=f"scA{c}_{kc}_{hl}")
                        scB = psA.tile([P, 512], f32, tag="ps", name=f"scB{c}_{kc}_{hl}")
                        nc.tensor.matmul(
                            scA[:],
                            lhsT=kt_sl,
                            rhs=qt[d0:d1, kc * P : kc * P + 512],
                            start=True, stop=True,
                        )
                        nc.tensor.matmul(
                            scB[:, 0 : W - 512],
                            lhsT=kt_sl,
                            rhs=qt[d0:d1, kc * P + 512 : S],
                            start=True, stop=True,
                        )
                        scs.append((scA, scB))
                    else:
                        scA = psA.tile([P, 512], f32, tag="ps", name=f"scA{c}_{kc}_{hl}")
                        nc.tensor.matmul(
                            scA[:, 0:W],
                            lhsT=kt_sl,
                            rhs=qt[d0:d1, kc * P : S],
                            start=True, stop=True,
                        )
                        scs.append((scA, None))
                for hl in range(2):
                    scA, scB = scs[hl]
                    at = at_pool.tile([P, W], bf16, tag="at", name=f"at{c}_{kc}_{hl}")
                    if scB is not None:
                        nc.scalar.activation(
                            at[:, 0:512], scA[:], EXP, bias=0.0, scale=float(SCALE)
                        )
                        nc.scalar.activation(
                            at[:, 512:W], scB[:, 0 : W - 512], EXP,
                            bias=0.0, scale=float(SCALE),
                        )
                    else:
                        nc.scalar.activation(
                            at[:, 0:W], scA[:, 0:W], EXP, bias=0.0, scale=float(SCALE)
                        )
                    st["at"][(hl, kc)] = at
                for hl in range(2):
                    at = st["at"][(hl, kc)]
                    # zero the strictly-lower triangle of the diagonal block
                    nc.vector.tensor_mul(at[:, 0:P], at[:, 0:P], binmaskT[:])

            def attn_v(c, hl, kc):
                st = state[c]
                if "po" not in st:
                    st["po"] = [
                        psB.tile([P, S], f32, tag="po", name=f"po{c}_{h}")
                        for h in range(2)
                    ]
                W = S - kc * P
                at = st["at"].pop((hl, kc))
                # h0 stationary = [v_h0 | ones]; h1 = [ones | v_h1]
                lhsT = (
                    v_sb[kc][:, c, 0 : 2 * D]
                    if hl == 0
                    else v_sb[kc][:, c, D : 3 * D]
                )
                p = st["po"][hl]
                if kc == 0:
                    nc.tensor.matmul(
                        p[:, 0:512], lhsT=lhsT, rhs=at[:, 0:512],
                        start=True, stop=False,
                    )
                    nc.tensor.matmul(
                        p[:, 512:1024], lhsT=lhsT, rhs=at[:, 512:1024],
                        start=True, stop=False,
                    )
                elif kc < 4:
                    # split on the aT exp_a/exp_b boundary (col 512) so
                    # each matmul depends on exactly one exp instruction
                    nc.tensor.matmul(
                        p[:, kc * P : 512], lhsT=lhsT, rhs=at[:, 0 : 512 - kc * P],
                        start=False, stop=(kc == 3),
                    )
                    nc.tensor.matmul(
                        p[:, 512 : 512 + kc * P],
                        lhsT=lhsT,
                        rhs=at[:, 512 - kc * P : 512],
                        start=False, stop=False,
                    )
                    nc.tensor.matmul(
                        p[:, 512 + kc * P : 1024],
                        lhsT=lhsT,
                        rhs=at[:, 512:W],
                        start=False, stop=False,
                    )
                else:
                    nc.tensor.matmul(
                        p[:, kc * P : 1024], lhsT=lhsT, rhs=at[:, 0:W],
                        start=False, stop=(kc == 7),
                    )

            def normalize(c):
                # both heads' replicated row-sums -> one SBUF tile,
                # r = exp(-ln(sums)) on ACT, then outT rows * r on DVE
                po = state[c]["po"]
                den = rbc_pool.tile([P, S], f32, tag="den", name=f"den{c}")
                rr = rbc_pool.tile([P, S], f32, tag="rr", name=f"rr{c}")
                nc.vector.tensor_copy(den[D : 2 * D, :], po[0][D : 2 * D, :])
                nc.vector.tensor_copy(den[0:D, :], po[1][0:D, :])
                nc.scalar.activation(rr[:], den[:], LOG)
                nc.scalar.activation(rr[:], rr[:], EXP, bias=0.0, scale=-1.0)
                nc.vector.tensor_mul(ct_sb[c][0:D, :], po[0][0:D, :], rr[D : 2 * D, :])
                nc.vector.tensor_mul(
                    ct_sb[c][D : 2 * D, :], po[1][D : 2 * D, :], rr[0:D, :]
                )
                del state[c]

            qt_next = qk_alloc(0)
            for j in range(4):
                qk_chunk(0, qt_next[0], qt_next[1], j)

            G = NPAIR * NT
            for g in range(G + LAG):
                if g < G:
                    c, kc = divmod(g, NT)
                    if kc == 0:
                        state[c] = dict(qt=qt_next[0], kt=qt_next[1], at={})
                    scores_iter(c, kc)
                    if c + 1 < NPAIR:
                        if kc == 2:
                            qt_next = qk_alloc(c + 1)
                        if 2 <= kc <= 5:
                            qk_chunk(c + 1, qt_next[0], qt_next[1], kc - 2)
                if g >= LAG:
                    c2, kc2 = divmod(g - LAG, NT)
                    attn_v(c2, 0, kc2)
                    attn_v(c2, 1, kc2)
                    if kc2 == NT - 1:
                        normalize(c2)

            # ---------- output projection + bias ----------
            for t in range(NT):
                osb = out_pool.tile([P, E], bf16, tag="out", name=f"ou{t}")
                for hf in range(2):
                    op = psA.tile([P, 512], f32, tag="ps", name=f"op{t}_{hf}")
                    for cc in range(NCH):
                        nc.tensor.matmul(
                            op[:],
                            lhsT=ct_sb[cc][:, t * P : (t + 1) * P],
                            rhs=wo_sb[cc][:, hf * 512 : (hf + 1) * 512],
                            start=(cc == 0),
                            stop=(cc == NCH - 1),
                        )
                    nc.vector.tensor_add(
                        osb[:, hf * 512 : (hf + 1) * 512],
                        op[:],
                        bo_sb[:, hf * 512 : (hf + 1) * 512],
                    )
                eng = nc.sync if t % 2 == 0 else nc.scalar
                eng.dma_start(out=out_ext[t * P : (t + 1) * P, :], in_=osb[:])

    return _split_waits(nc)


def _get_graph():
    if "nc" not in _graph_cache:
        _patch_tile_drain()
        _graph_cache["nc"] = _build()
    return _graph_cache["nc"]


def _prep_inputs(x, Wq, Wk, Wv, Wo, bo):
    xT = np.ascontiguousarray(np.transpose(np.asarray(x, np.float32), (0, 2, 1)))
    xT = xT.astype(BF16)
    wq = np.ascontiguousarray(
        np.asarray(Wq, np.float32).transpose(1, 0, 2).reshape(E, HD)
    ).astype(BF16)
    wk = np.ascontiguousarray(
        np.asarray(Wk, np.float32).transpose(1, 0, 2).reshape(E, HD)
    ).astype(BF16)
    wv = np.ascontiguousarray(
        np.asarray(Wv, np.float32).transpose(1, 0, 2).reshape(E, HD)
    ).astype(BF16)
    wo = np.ascontiguousarray(np.asarray(Wo, np.float32)).astype(BF16)
    bo_t = np.ascontiguousarray(
        np.tile(np.asarray(bo, np.float32)[None, :], (P, 1))
    )
    return [
        dict(
            xT=np.ascontiguousarray(xT[b]),
            wq=wq,
            wk=wk,
            wv=wv,
            wo=wo,
            bo=bo_t,
        )
        for b in range(B)
    ]


def _run(in_maps, **kw):
    from concourse.bass_utils import run_bass_kernel_spmd

    nc = _get_graph()
    return run_bass_kernel_spmd(nc, in_maps, core_ids=list(range(NCORES)), **kw)


def kernel(x, Wq, Wk, Wv, Wo, bo):
    res = _run(_prep_inputs(x, Wq, Wk, Wv, Wo, bo))
    return np.stack(
        [np.asarray(res.results[b]["out"], np.float32) for b in range(B)], axis=0
    )


# revision 25
# speedup vs baseline: 1.3607x; 1.1220x over previous
"""Causal multi-head attention (B=8, S=1024, E=1024, H=16, D=64) on 8 TRN2 NeuronCores.

Strategy: pure data parallelism over the batch — one batch element per core,
full weights replicated, zero collectives. Per-core attention with TRANSPOSED
scores (k on partitions), which eliminates the DMA-crossbar transpose of the
attention matrix entirely:

  - host passes x[b] pre-transposed (xT = [E, S]) and weights cast to bf16;
    all matmuls run in bf16 with fp32 PSUM accumulation.
  - V = [s, pair*(64+64+64)] is projected first; per pair the layout is
    [v_h0 | ones | v_h1] so head h0's attn@V stationary [v_h0|ones] puts the
    softmax row-sums (replicated 64x) in PSUM rows 64:128, and h1's
    [ones|v_h1] puts them in rows 0:64 — the sums come free with the matmul.
  - QT/KT = [head*64+d, s] chunks are projected just-in-time per head-pair;
    their PSUM->SBUF copies run on ACT so DVE stays off that chain.
  - scoresT[k, q] per (head, k-chunk): lhsT = KT chunk (stationary), rhs
    streams all causal q columns (up to 512/instr). The causal mask for the
    diagonal block is INJECTED INTO PSUM by an identity matmul (start=True)
    that the scores matmul then accumulates onto (start=False) — masking
    costs one extra 128-col PE matmul, no DVE/ACT work.
  - exp on ACT reads the PSUM scores and writes unnormalized attnT tiles
    (bf16, exact causal widths, no transpose needed).
  - attn@V accumulates outT = [d, q] over k-chunks with wide streams;
    normalization is deferred: one reciprocal_approx_fast + one [64, S]
    multiply per head (vs. normalizing the full [128, 4608] attn matrix).
  - out = ct.T @ Wo + bias-add on DVE from a host-broadcast bo tile.
"""

import numpy as np
import ml_dtypes

B, S, E = 8, 1024, 1024
H, D = 16, 64
HD = H * D
NCORES = 8
P = 128
NCH = E // P  # 8 contraction chunks
NT = S // P  # 8 q tiles
NPAIR = H // 2
SCALE = 1.0 / np.sqrt(D)
BF16 = ml_dtypes.bfloat16

_graph_cache = {}


def _patch_tile_drain():
    """The walrus build in this container only allows a single sync wait on the
    TPB_CTRL Drain that TileContext emits at kernel tail. Spread the end-of-
    kernel waits across SP nops (one wait each) before the drain instead."""
    import concourse.tile as tile
    import concourse.mybir as mybir
    from concourse.vector_clock import ScopedClock

    if getattr(tile.TileContext, "_drain_patched", False):
        return

    def _drain_and_barrier(self, tick_clock, wait_clock):
        nop0 = self.nc.sync.nop(nofuse=True)
        wait_clock.add_sem_waits(
            nop0.ins, ScopedClock({None: tick_clock.global_clock})
        )
        waits = list(nop0.ins.sync_info.on_wait) if nop0.ins.sync_info else []
        if len(waits) > 1:
            nop0.ins.sync_info = mybir.SyncInfo(
                on_wait=waits[:1], on_update=list(nop0.ins.sync_info.on_update)
            )
            for w in waits[1:]:
                n = self.nc.sync.nop(nofuse=True)
                n.ins.sync_info = mybir.SyncInfo(on_wait=[w], on_update=[])
        self.nc.sync.drain()
        self.nc.all_engine_barrier()
        assert self.sems is not None
        popped = self.nc._tile_sem_poison_stack.pop()
        assert popped is self._sem_poison
        self.nc.clear_and_free_semaphores(list(self.sems.allocated().values()))
        self.nc.all_engine_barrier()

    tile.TileContext._drain_and_barrier = _drain_and_barrier
    tile.TileContext._drain_patched = True


def _split_waits(nc, maxw=1):
    """This container's walrus build rejects instructions carrying more than
    one sync wait. Move excess waits onto same-engine NoOps inserted directly
    before the instruction (sequencer order makes this semantically identical.
    Safe for a straight-line Tile program: waits only reference predecessors
    in the dependency DAG, so stalling the sequencer earlier cannot deadlock."""
    import concourse.mybir as mybir

    ctr = [0]
    for fn in nc.m.functions:
        for bb in fn.blocks:
            insts = bb.instructions
            out = []
            changed = False
            for inst in insts:
                si = getattr(inst, "sync_info", None)
                if si is not None and len(si.on_wait) > maxw:
                    waits = list(si.on_wait)
                    keep = waits[:maxw]
                    rest = waits[maxw:]
                    for i in range(0, len(rest), maxw):
                        n = mybir.InstNoOp(
                            name=f"waitsplit-{ctr[0]}", ins=[], outs=[]
                        )
                        ctr[0] += 1
                        n.engine = inst.engine
                        n.sync_info = mybir.SyncInfo(
                            on_wait=rest[i : i + maxw], on_update=[]
                        )
                        out.append(n)
                    inst.sync_info = mybir.SyncInfo(
                        on_wait=keep, on_update=list(si.on_update)
                    )
                    changed = True
                out.append(inst)
            if changed:
                bb.instructions = out
    return nc


def _build():
    import concourse.bass as bass
    import concourse.tile as tile
    import concourse.mybir as mybir

    f32 = mybir.dt.float32
    bf16 = mybir.dt.bfloat16

    nc = bass.Bass("TRN2", target_bir_lowering=False, debug=False, num_devices=NCORES)
    xT_ext = nc.declare_dram_parameter("xT", [E, S], bf16, isOutput=False)
    wq_ext = nc.declare_dram_parameter("wq", [E, HD], bf16, isOutput=False)
    wk_ext = nc.declare_dram_parameter("wk", [E, HD], bf16, isOutput=False)
    wv_ext = nc.declare_dram_parameter("wv", [E, HD], bf16, isOutput=False)
    wo_ext = nc.declare_dram_parameter("wo", [HD, E], bf16, isOutput=False)
    bo_ext = nc.declare_dram_parameter("bo", [P, E], f32, isOutput=False)
    out_ext = nc.declare_dram_parameter("out", [S, E], bf16, isOutput=True)

    EXP = mybir.ActivationFunctionType.Exp
    LOG = mybir.ActivationFunctionType.Ln

    with tile.TileContext(nc) as tc:
        with (
            tc.tile_pool(name="consts", bufs=1) as consts,
            tc.tile_pool(name="xt", bufs=NCH) as xt_pool,
            tc.tile_pool(name="w3", bufs=3 * NCH) as w3_pool,
            tc.tile_pool(name="wo", bufs=NCH) as wo_pool,
            tc.tile_pool(name="vp", bufs=NT) as v_pool,
            tc.tile_pool(name="qt", bufs=2) as qt_pool,
            tc.tile_pool(name="kt", bufs=2) as kt_pool,
            tc.tile_pool(name="ct", bufs=NCH) as ct_pool,
            tc.tile_pool(name="at", bufs=8) as at_pool,
            tc.tile_pool(name="rbc", bufs=4) as rbc_pool,
            tc.tile_pool(name="outp", bufs=2) as out_pool,
            tc.tile_pool(name="psA", bufs=4, space="PSUM") as psA,
            tc.tile_pool(name="psB", bufs=2, space="PSUM") as psB,
        ):
            # constants: transposed binary causal mask (1 where q >= k,
            # 0 strictly below the diagonal), broadcast bias
            binmaskT = consts.tile([P, P], bf16, tag="binmaskT")
            nc.gpsimd.memset(binmaskT[:], 1.0)
            nc.gpsimd.affine_select(
                out=binmaskT[:],
                in_=binmaskT[:],
                compare_op=mybir.AluOpType.is_ge,
                fill=0.0,
                base=0,
                # keep 1 where (q - k) >= 0, fill 0 where q < k
                pattern=[[1, P]],
                channel_multiplier=-1,
            )
            bo_sb = consts.tile([P, E], f32, tag="bo")

            xt_sb = [xt_pool.tile([P, S], bf16, tag="xt", name=f"xt{_}") for _ in range(NCH)]
            wq_sb = [w3_pool.tile([P, HD], bf16, tag="w3", name=f"wq{_}") for _ in range(NCH)]
            wk_sb = [w3_pool.tile([P, HD], bf16, tag="w3", name=f"wk{_}") for _ in range(NCH)]
            wv_sb = [w3_pool.tile([P, HD], bf16, tag="w3", name=f"wv{_}") for _ in range(NCH)]
            wo_sb = [wo_pool.tile([P, E], bf16, tag="wo", name=f"wo{_}") for _ in range(NCH)]
            # x + Wv first (V projection is the critical path at start),
            # then Wq/Wk (needed from pair 0 on), Wo last.
            for c in range(NCH):
                rows = slice(c * P, (c + 1) * P)
                nc.sync.dma_start(out=xt_sb[c][:], in_=xT_ext[rows, :])
                nc.scalar.dma_start(out=wv_sb[c][:], in_=wv_ext[rows, :])
            for c in range(NCH):
                rows = slice(c * P, (c + 1) * P)
                nc.sync.dma_start(out=wq_sb[c][:], in_=wq_ext[rows, :])
                nc.scalar.dma_start(out=wk_sb[c][:], in_=wk_ext[rows, :])
            for c in range(NCH):
                rows = slice(c * P, (c + 1) * P)
                eng = nc.sync if c % 2 == 0 else nc.scalar
                eng.dma_start(out=wo_sb[c][:], in_=wo_ext[rows, :])
            nc.scalar.dma_start(out=bo_sb[:], in_=bo_ext[:])

            # ---------- V projection; per-pair layout [v_h0 | ones | v_h1] ----
            v_sb = [
                v_pool.tile([P, NPAIR, 3 * D], bf16, tag="v", name=f"v{_}")
                for _ in range(NT)
            ]
            for t in range(NT):
                for hf in range(2):
                    ps = psA.tile([P, 512], f32, tag="ps", name=f"vps{t}_{hf}")
                    for k in range(NCH):
                        nc.tensor.matmul(
                            ps[:],
                            lhsT=xt_sb[k][:, t * P : (t + 1) * P],
                            rhs=wv_sb[k][:, hf * 512 : (hf + 1) * 512],
                            start=(k == 0),
                            stop=(k == NCH - 1),
                        )
                    ps3 = ps[:].rearrange("p (g c) -> p g c", c=2 * D)
                    gsl = slice(hf * 4, (hf + 1) * 4)
                    nc.vector.tensor_copy(v_sb[t][:, gsl, 0:D], ps3[:, :, 0:D])
                    nc.vector.tensor_copy(
                        v_sb[t][:, gsl, 2 * D : 3 * D], ps3[:, :, D : 2 * D]
                    )
                nc.gpsimd.memset(v_sb[t][:, :, D : 2 * D], 1.0)

            ct_sb = [ct_pool.tile([P, S], bf16, tag="ct", name=f"ct{_}") for _ in range(NCH)]

            # ---------- per head-pair attention ----------
            def qk_alloc(c):
                qt = qt_pool.tile([P, S], bf16, tag="qt", name=f"qt{c}")
                kt = kt_pool.tile([P, S], bf16, tag="kt", name=f"kt{c}")
                return qt, kt

            def qk_chunk(c, qt, kt, j):
                # one of the four (projection, s-half) psum tiles: 8 matmuls
                # + its copy — sized to interleave between kc iterations
                w_sb, dst = ((wq_sb, qt), (wk_sb, kt))[j // 2]
                hf = j % 2
                ps = psA.tile([P, 512], f32, tag="ps", name=f"qk{c}_{j}")
                for k in range(NCH):
                    nc.tensor.matmul(
                        ps[:],
                        lhsT=w_sb[k][:, c * P : (c + 1) * P],
                        rhs=xt_sb[k][:, hf * 512 : (hf + 1) * 512],
                        start=(k == 0),
                        stop=(k == NCH - 1),
                    )
                nc.vector.tensor_copy(dst[:, hf * 512 : (hf + 1) * 512], ps[:])

            # Cross-pair software pipeline: one global stream of (pair, kc)
            # iterations; attn@V lags scores by LAG iterations and flows
            # across pair boundaries so the PE never drains (a drained PE
            # drops to half clock and takes microseconds to ramp back).
            LAG = 2
            state = {}

            def scores_iter(c, kc):
                st = state[c]
                qt, kt = st["qt"], st["kt"]
                W = S - kc * P
                scs = []
                for hl in range(2):
                    d0, d1 = hl * D, (hl + 1) * D
                    kt_sl = kt[d0:d1, kc * P : (kc + 1) * P]
                    if W > 512:
                        scA = psA.tile([P, 512], f32, tag="ps", name=f"scA{c}_{kc}_{hl}")
                        scB = psA.tile([P, 512], f32, tag="ps", name=f"scB{c}_{kc}_{hl}")
                        nc.tensor.matmul(
                            scA[:],
                            lhsT=kt_sl,
                            rhs=qt[d0:d1, kc * P : kc * P + 512],
                            start=True, stop=True,
                        )
                        nc.tensor.matmul(
                            scB[:, 0 : W - 512],
                            lhsT=kt_sl,
                            rhs=qt[d0:d1, kc * P + 512 : S],
                            start=True, stop=True,
                        )
                        scs.append((scA, scB))
                    else:
                        scA = psA.tile([P, 512], f32, tag="ps", name=f"scA{c}_{kc}_{hl}")
                        nc.tensor.matmul(
                            scA[:, 0:W],
                            lhsT=kt_sl,
                            rhs=qt[d0:d1, kc * P : S],
                            start=True, stop=True,
                        )
                        scs.append((scA, None))
                for hl in range(2):
                    scA, scB = scs[hl]
                    at = at_pool.tile([P, W], bf16, tag="at", name=f"at{c}_{kc}_{hl}")
                    if scB is not None:
                        nc.scalar.activation(
                            at[:, 0:512], scA[:], EXP, bias=0.0, scale=float(SCALE)
                        )
                        nc.scalar.activation(
                            at[:, 512:W], scB[:, 0 : W - 512], EXP,
                            bias=0.0, scale=float(SCALE),
                        )
                    else:
                        nc.scalar.activation(
                            at[:, 0:W], scA[:, 0:W], EXP, bias=0.0, scale=float(SCALE)
                        )
                    st["at"][(hl, kc)] = at
                for hl in range(2):
                    at = st["at"][(hl, kc)]
                    # zero the strictly-lower triangle of the diagonal block
                    nc.vector.tensor_mul(at[:, 0:P], at[:, 0:P], binmaskT[:])

            def attn_v(c, hl, kc):
                st = state[c]
                if "po" not in st:
                    st["po"] = [
                        psB.tile([P, S], f32, tag="po", name=f"po{c}_{h}")
                        for h in range(2)
                    ]
                W = S - kc * P
                at = st["at"].pop((hl, kc))
                # h0 stationary = [v_h0 | ones]; h1 = [ones | v_h1]
                lhsT = (
                    v_sb[kc][:, c, 0 : 2 * D]
                    if hl == 0
                    else v_sb[kc][:, c, D : 3 * D]
                )
                p = st["po"][hl]
                if kc == 0:
                    nc.tensor.matmul(
                        p[:, 0:512], lhsT=lhsT, rhs=at[:, 0:512],
                        start=True, stop=False,
                    )
                    nc.tensor.matmul(
                        p[:, 512:1024], lhsT=lhsT, rhs=at[:, 512:1024],
                        start=True, stop=False,
                    )
                elif kc < 4:
                    # split on the aT exp_a/exp_b boundary (col 512) so
                    # each matmul depends on exactly one exp instruction
                    nc.tensor.matmul(
                        p[:, kc * P : 512], lhsT=lhsT, rhs=at[:, 0 : 512 - kc * P],
                        start=False, stop=(kc == 3),
                    )
                    nc.tensor.matmul(
                        p[:, 512 : 512 + kc * P],
                        lhsT=lhsT,
                        rhs=at[:, 512 - kc * P : 512],
                        start=False, stop=False,
                    )
                    nc.tensor.matmul(
                        p[:, 512 + kc * P : 1024],
                        lhsT=lhsT,
                        rhs=at[:, 512:W],
                        start=False, stop=False,
                    )
                else:
                    nc.tensor.matmul(
                        p[:, kc * P : 1024], lhsT=lhsT, rhs=at[:, 0:W],
                        start=False, stop=(kc == 7),
                    )

            def normalize(c):
                # both heads' replicated row-sums -> one SBUF tile,
                # r = exp(-ln(sums)) on ACT, then outT rows * r on DVE
                po = state[c]["po"]
                den = rbc_pool.tile([P, S], f32, tag="den", name=f"den{c}")
                rr = rbc_pool.tile([P, S], f32, tag="rr", name=f"rr{c}")
                nc.vector.tensor_copy(den[D : 2 * D, :], po[0][D : 2 * D, :])
                nc.vector.tensor_copy(den[0:D, :], po[1][0:D, :])
                nc.scalar.activation(rr[:], den[:], LOG)
                nc.scalar.activation(rr[:], rr[:], EXP, bias=0.0, scale=-1.0)
                nc.vector.tensor_mul(ct_sb[c][0:D, :], po[0][0:D, :], rr[D : 2 * D, :])
                nc.vector.tensor_mul(
                    ct_sb[c][D : 2 * D, :], po[1][D : 2 * D, :], rr[0:D, :]
                )
                del state[c]

            qt_next = qk_alloc(0)
            for j in range(4):
                qk_chunk(0, qt_next[0], qt_next[1], j)

            G = NPAIR * NT
            for g in range(G + LAG):
                if g < G:
                    c, kc = divmod(g, NT)
                    if kc == 0:
                        state[c] = dict(qt=qt_next[0], kt=qt_next[1], at={})
                    scores_iter(c, kc)
                    if c + 1 < NPAIR:
                        if kc == 0:
                            qt_next = qk_alloc(c + 1)
                        if kc <= 3:
                            qk_chunk(c + 1, qt_next[0], qt_next[1], kc)
                if g >= LAG:
                    c2, kc2 = divmod(g - LAG, NT)
                    attn_v(c2, 0, kc2)
                    attn_v(c2, 1, kc2)
                    if kc2 == NT - 1:
                        normalize(c2)

            # ---------- output projection + bias ----------
            for t in range(NT):
                osb = out_pool.tile([P, E], bf16, tag="out", name=f"ou{t}")
                for hf in range(2):
                    op = psA.tile([P, 512], f32, tag="ps", name=f"op{t}_{hf}")
                    for cc in range(NCH):
                        nc.tensor.matmul(
                            op[:],
                            lhsT=ct_sb[cc][:, t * P : (t + 1) * P],
                            rhs=wo_sb[cc][:, hf * 512 : (hf + 1) * 512],
                            start=(cc == 0),
                            stop=(cc == NCH - 1),
                        )
                    nc.vector.tensor_add(
                        osb[:, hf * 512 : (hf + 1) * 512],
                        op[:],
                        bo_sb[:, hf * 512 : (hf + 1) * 512],
                    )
                eng = nc.sync if t % 2 == 0 else nc.scalar
                eng.dma_start(out=out_ext[t * P : (t + 1) * P, :], in_=osb[:])

    return _split_waits(nc)


def _get_graph():
    if "nc" not in _graph_cache:
        _patch_tile_drain()
        _graph_cache["nc"] = _build()
    return _graph_cache["nc"]


def _prep_inputs(x, Wq, Wk, Wv, Wo, bo):
    xT = np.ascontiguousarray(np.transpose(np.asarray(x, np.float32), (0, 2, 1)))
    xT = xT.astype(BF16)
    wq = np.ascontiguousarray(
        np.asarray(Wq, np.float32).transpose(1, 0, 2).reshape(E, HD)
    ).astype(BF16)
    wk = np.ascontiguousarray(
        np.asarray(Wk, np.float32).transpose(1, 0, 2).reshape(E, HD)
    ).astype(BF16)
    wv = np.ascontiguousarray(
        np.asarray(Wv, np.float32).transpose(1, 0, 2).reshape(E, HD)
    ).astype(BF16)
    wo = np.ascontiguousarray(np.asarray(Wo, np.float32)).astype(BF16)
    bo_t = np.ascontiguousarray(
        np.tile(np.asarray(bo, np.float32)[None, :], (P, 1))
    )
    return [
        dict(
            xT=np.ascontiguousarray(xT[b]),
            wq=wq,
            wk=wk,
            wv=wv,
            wo=wo,
            bo=bo_t,
        )
        for b in range(B)
    ]


def _run(in_maps, **kw):
    from concourse.bass_utils import run_bass_kernel_spmd

    nc = _get_graph()
    return run_bass_kernel_spmd(nc, in_maps, core_ids=list(range(NCORES)), **kw)


def kernel(x, Wq, Wk, Wv, Wo, bo):
    res = _run(_prep_inputs(x, Wq, Wk, Wv, Wo, bo))
    return np.stack(
        [np.asarray(res.results[b]["out"], np.float32) for b in range(B)], axis=0
    )
